# revision 7
# baseline (speedup 1.0000x reference)
"""Block-causal attention kernel for Trainium2, 8 NeuronCores.

Sharding: core c in 0..7 handles batch b = c//4 and 4 heads starting at
(c%4)*4.  Each core computes RMSNorm, QKV projection (fp32r / TF32 matmuls),
interleaved RoPE, block-causal attention, and a partial output projection over
its 4 heads.  Host sums the 4 partial y per batch and stacks v.

Self-contained: hardcodes shapes from the problem spec.
"""
import sys

sys.path.insert(0, "/opt/trn_rl_repo")

import numpy as np

import concourse.bacc as bacc
import concourse.tile as tile
from concourse import mybir
from concourse.bass_utils import run_bass_kernel_spmd

B, N, DIM = 2, 2048, 1024
HEADS, DHEAD, BLOCK = 16, 64, 128
THETA = 10000.0
P = 128
ST = N // P          # 16 seq tiles
KT = DIM // P        # 8 contraction tiles
NH = HEADS // 8 * 2  # 4 heads per core (wait: 16 heads / 4 groups)
NH = 4
SCALE = DHEAD ** -0.5
EPS = float(np.finfo(np.float32).eps)

f32 = mybir.dt.float32
f32r = mybir.dt.float32r
AF = mybir.ActivationFunctionType

PAIR_SWAP = [m ^ 1 for m in range(32)]


def _to_fp32r(x: np.ndarray) -> np.ndarray:
    """Round fp32 to tf32 (11-bit mantissa) the way the PE consumes it."""
    u = np.ascontiguousarray(x, dtype=np.float32).view(np.uint32)
    return (((u.astype(np.uint64) + 0x800) & 0xFFFFF000).astype(np.uint32)).view(
        np.float32
    )


def _emit(nc, tc, ctx):
    from contextlib import ExitStack

    x_d = nc.declare_dram_parameter("x", [N, DIM], f32, isOutput=False)
    wqk_d = nc.declare_dram_parameter("wqk", [DIM, NH * 128], f32r, isOutput=False)
    wv_d = nc.declare_dram_parameter("wv", [DIM, NH * 64], f32r, isOutput=False)
    wout_d = nc.declare_dram_parameter("wout", [NH * 64, DIM], f32r, isOutput=False)
    cos_d = nc.declare_dram_parameter("cosT", [P, N], f32, isOutput=False)
    sin2_d = nc.declare_dram_parameter("sinT2", [P, N], f32, isOutput=False)
    id_d = nc.declare_dram_parameter("ident", [P, P], f32r, isOutput=False)
    y_d = nc.declare_dram_parameter("y", [N, DIM], f32, isOutput=True)
    v_d = nc.declare_dram_parameter("vout", [N, NH, DHEAD], f32, isOutput=True)

    persist = ctx.enter_context(tc.tile_pool(name="persist", bufs=1))
    v_sb = persist.tile([P, ST, NH, DHEAD + 1], f32r)
    qq = [persist.tile([P, N], f32r, tag=f"qq{p}", name=f"qq{p}") for p in range(2)]
    kk = [persist.tile([P, N], f32r, tag=f"kk{p}", name=f"kk{p}") for p in range(2)]
    eps_sb = persist.tile([P, 1], f32)
    nc.vector.memset(eps_sb[:], EPS)
    one_sb = persist.tile([P, 1], f32)
    nc.vector.memset(one_sb[:], 1.0)

    with ExitStack() as early:
        epool = early.enter_context(tc.tile_pool(name="epool", bufs=1))
        work = early.enter_context(tc.tile_pool(name="work", bufs=3))
        rope = early.enter_context(tc.tile_pool(name="rope", bufs=2))
        stat = early.enter_context(tc.tile_pool(name="stat", bufs=4))
        ps_tp = early.enter_context(tc.tile_pool(name="ps_tp", bufs=2, space="PSUM"))
        ps_qk = early.enter_context(tc.tile_pool(name="ps_qk", bufs=1, space="PSUM"))
        ps_v = early.enter_context(tc.tile_pool(name="ps_v", bufs=2, space="PSUM"))

        wqk_sb = epool.tile([P, KT, NH * 128], f32r)
        wv_sb = epool.tile([P, KT, NH * 64], f32r)
        cos_sb = epool.tile([P, N], f32)
        sin2_sb = epool.tile([P, N], f32)
        id_sb = epool.tile([P, P], f32r)
        xnT = epool.tile([P, KT, N], f32r)

        for kt in range(KT):
            nc.sync.dma_start(out=wqk_sb[:, kt, :], in_=wqk_d[kt * P:(kt + 1) * P, :])
            nc.sync.dma_start(out=wv_sb[:, kt, :], in_=wv_d[kt * P:(kt + 1) * P, :])
        nc.sync.dma_start(out=cos_sb[:], in_=cos_d[:])
        nc.sync.dma_start(out=sin2_sb[:], in_=sin2_d[:])
        nc.sync.dma_start(out=id_sb[:], in_=id_d[:])

        # ---- phase 1: rmsnorm + transpose -> xnT --------------------------
        for s in range(ST):
            x_t = work.tile([P, DIM], f32, tag="x")
            nc.sync.dma_start(out=x_t[:], in_=x_d[s * P:(s + 1) * P, :])
            sq = work.tile([P, DIM], f32, tag="sq", bufs=2)
            ss = stat.tile([P, 1], f32, tag="ss")
            nc.scalar.activation(out=sq[:], in_=x_t[:], func=AF.Square,
                                 accum_out=ss[:])
            rstd = stat.tile([P, 1], f32, tag="rstd")
            nc.scalar.activation(out=rstd[:], in_=ss[:], func=AF.Sqrt,
                                 bias=eps_sb[:], scale=1.0 / DIM)
            nc.vector.reciprocal(out=rstd[:], in_=rstd[:])
            xn = work.tile([P, DIM], f32r, tag="xn")
            nc.scalar.activation(out=xn[:], in_=x_t[:], func=AF.Copy,
                                 scale=rstd[:])
            for g in range(2):
                pst = ps_tp.tile([P, 4, P], f32r, tag="pst")
                for dd in range(4):
                    d = g * 4 + dd
                    nc.tensor.transpose(pst[:, dd, :], xn[:, d * P:(d + 1) * P],
                                        id_sb[:])
                nc.scalar.copy(
                    out=xnT[:, g * 4:(g + 1) * 4, s * P:(s + 1) * P], in_=pst[:])

        # ---- phase 2: v projection ---------------------------------------
        for s in range(ST):
            pv = ps_v.tile([P, NH * 64], f32, tag="pv")
            for kt in range(KT):
                nc.tensor.matmul(
                    pv[:], xnT[:, kt, s * P:(s + 1) * P], wv_sb[:, kt, :],
                    start=(kt == 0), stop=(kt == KT - 1))
            vo = work.tile([P, NH, DHEAD], f32, tag="vo", bufs=2)
            nc.scalar.copy(out=vo[:], in_=pv[:].rearrange("p (h d) -> p h d", h=NH))
            nc.sync.dma_start(out=v_d[s * P:(s + 1) * P, :, :], in_=vo[:])
            nc.vector.tensor_copy(
                out=v_sb[:, s, :, 0:DHEAD],
                in_=pv[:].rearrange("p (h d) -> p h d", h=NH))
            nc.vector.tensor_copy(out=v_sb[:, s, :, DHEAD],
                                  in_=one_sb[:].to_broadcast((P, NH)))

        # ---- phase 3: qk projection + rope -------------------------------
        for h in range(NH):
            ro = (h % 2) * 64
            pl = h // 2
            for sh in range(2):
                pqk = ps_qk.tile([P, N // 2], f32, tag="pqk")
                for kt in range(KT):
                    for nch in range(2):
                        c0 = sh * (N // 2) + nch * 512
                        nc.tensor.matmul(
                            pqk[:, nch * 512:(nch + 1) * 512],
                            wqk_sb[:, kt, h * 128:(h + 1) * 128],
                            xnT[:, kt, c0:c0 + 512],
                            start=(kt == 0), stop=(kt == KT - 1))
                for nch in range(2):
                    col = sh * (N // 2) + nch * 512
                    pq = pqk[:, nch * 512:(nch + 1) * 512]
                    a_t = rope.tile([P, 512], f32, tag="ropea")
                    nc.vector.tensor_mul(a_t[:], pq, sin2_sb[:, col:col + 512])
                    b_t = rope.tile([P, 512], f32, tag="ropeb")
                    nc.vector.stream_shuffle(b_t[:], a_t[:], PAIR_SWAP)
                    c_t = rope.tile([P, 512], f32, tag="ropec")
                    nc.vector.tensor_mul(c_t[:], pq, cos_sb[:, col:col + 512])
                    nc.vector.tensor_add(
                        qq[pl][ro:ro + 64, col:col + 512], c_t[0:64, :],
                        b_t[0:64, :])
                    nc.vector.tensor_add(
                        kk[pl][ro:ro + 64, col:col + 512], c_t[64:128, :],
                        b_t[64:128, :])

    # ---- phase 4: attention + phase 5: output projection ------------------
    with ExitStack() as late:
        lpool = late.enter_context(tc.tile_pool(name="lpool", bufs=1))
        expp = late.enter_context(tc.tile_pool(name="expp", bufs=3))
        lstat = late.enter_context(tc.tile_pool(name="lstat", bufs=2))
        yout = late.enter_context(tc.tile_pool(name="yout", bufs=2))
        ps_sim = late.enter_context(tc.tile_pool(name="ps_sim", bufs=2,
                                                 space="PSUM"))
        ps_o = late.enter_context(tc.tile_pool(name="ps_o", bufs=1, space="PSUM"))
        ps_y = late.enter_context(tc.tile_pool(name="ps_y", bufs=2, space="PSUM"))
        dscratch = late.enter_context(tc.tile_pool(name="dscratch", bufs=2,
                                                   space="DRAM"))

        aoT = lpool.tile([P, 2, N], f32r)
        wout_sb = lpool.tile([P, 2, DIM], f32r)
        for kp in range(2):
            nc.sync.dma_start(out=wout_sb[:, kp, :],
                              in_=wout_d[kp * P:(kp + 1) * P, :])

        for h in range(NH):
            ro = (h % 2) * 64
            pl = h // 2
            for half in range(2):
                po = ps_o.tile([DHEAD + 1, N // 2], f32, tag="po")
                for j in range(8 * half + 8):
                    for c in range(max(j // 4, 2 * half), 2 * half + 2):
                        i0 = max(j * P, c * 512)
                        i1 = (c + 1) * 512
                        nw = i1 - i0
                        psim = ps_sim.tile([P, 512], f32, tag="psim")
                        nc.tensor.matmul(
                            psim[:, 0:nw],
                            kk[pl][ro:ro + 64, j * P:(j + 1) * P],
                            qq[pl][ro:ro + 64, i0:i1],
                            start=True, stop=True)
                        et = expp.tile([P, 512], f32r, tag="et")
                        nc.scalar.activation(out=et[:, 0:nw], in_=psim[:, 0:nw],
                                             func=AF.Exp)
                        nc.tensor.matmul(
                            po[:, i0 - half * (N // 2):i1 - half * (N // 2)],
                            v_sb[:, j, h, :],
                            et[:, 0:nw],
                            start=(j == 0), stop=(j == 4 * c + 3))
                s_sb = lstat.tile([1, N // 2], f32, tag="ssum")
                nc.scalar.copy(out=s_sb[:], in_=po[DHEAD:DHEAD + 1, :])
                r_sb = lstat.tile([1, N // 2], f32, tag="rsum")
                nc.vector.reciprocal_approx_fast(out=r_sb[:], in_=s_sb[:])
                r_dr = dscratch.tile([1, N // 2], f32, tag="rdr")
                nc.sync.dma_start(out=r_dr[:], in_=r_sb[:])
                bc = lstat.tile([64, N // 2], f32, tag="bcast")
                nc.sync.dma_start(out=bc[:],
                                  in_=r_dr[:].to_broadcast((64, N // 2)))
                nc.vector.tensor_mul(
                    aoT[ro:ro + 64, pl,
                        half * (N // 2):(half + 1) * (N // 2)],
                    po[0:DHEAD, :], bc[:])

        # ---- phase 5: output projection ----------------------------------
        for s in range(ST):
            ys = yout.tile([P, DIM], f32, tag="ys")
            for nh2 in range(2):
                py = ps_y.tile([P, 512], f32, tag="py")
                for kp in range(2):
                    nc.tensor.matmul(
                        py[:],
                        aoT[:, kp, s * P:(s + 1) * P],
                        wout_sb[:, kp, nh2 * 512:(nh2 + 1) * 512],
                        start=(kp == 0), stop=(kp == 1))
                nc.vector.tensor_copy(out=ys[:, nh2 * 512:(nh2 + 1) * 512],
                                      in_=py[:])
            nc.sync.dma_start(out=y_d[s * P:(s + 1) * P, :], in_=ys[:])


_NC = None


def _build():
    global _NC
    if _NC is None:
        from contextlib import ExitStack

        nc = bacc.Bacc("TRN2", target_bir_lowering=False, debug=False)
        with tile.TileContext(nc) as tc:
            with ExitStack() as ctx:
                _emit(nc, tc, ctx)
        nc.finalize()
        _NC = nc
    return _NC


def _host_tables():
    inv_freq = 1.0 / (THETA ** (np.arange(0, DHEAD, 2, dtype=np.float64) / DHEAD))
    pos = np.arange(N, dtype=np.float64)
    fr = pos[:, None] * inv_freq[None, :]          # [N, 32]
    fr = np.repeat(fr, 2, axis=-1)                 # [N, 64]
    cosd = np.cos(fr).T.astype(np.float32)         # [64, N]
    sind = np.sin(fr).T.astype(np.float32)
    sgn = np.where(np.arange(DHEAD) % 2 == 0, 1.0, -1.0).astype(np.float32)[:, None]
    cosT = np.concatenate([cosd * SCALE, cosd], axis=0)          # [128, N]
    sinT2 = np.concatenate([sind * sgn * SCALE, sind * sgn], axis=0)
    return cosT.astype(np.float32), sinT2.astype(np.float32)


def kernel(x, norm_w, w_qkv, w_out):
    x = np.asarray(x, dtype=np.float32)
    norm_w = np.asarray(norm_w, dtype=np.float32)
    w_qkv = np.asarray(w_qkv, dtype=np.float32)
    w_out = np.asarray(w_out, dtype=np.float32)

    nc = _build()
    cosT, sinT2 = _host_tables()
    ident = _to_fp32r(np.eye(P, dtype=np.float32))

    w_eff = w_qkv * norm_w[:, None]
    wq = w_eff[:, 0:HEADS * DHEAD]
    wk = w_eff[:, HEADS * DHEAD:2 * HEADS * DHEAD]
    wv = w_eff[:, 2 * HEADS * DHEAD:]

    in_maps = []
    for c in range(8):
        b, hg = c // 4, c % 4
        hs = [hg * NH + hh for hh in range(NH)]
        wqk_c = np.empty((DIM, NH * 128), np.float32)
        wv_c = np.empty((DIM, NH * 64), np.float32)
        wout_c = np.empty((NH * 64, DIM), np.float32)
        for hh, h in enumerate(hs):
            wqk_c[:, hh * 128:hh * 128 + 64] = wq[:, h * 64:(h + 1) * 64]
            wqk_c[:, hh * 128 + 64:hh * 128 + 128] = wk[:, h * 64:(h + 1) * 64]
            wv_c[:, hh * 64:(hh + 1) * 64] = wv[:, h * 64:(h + 1) * 64]
            wout_c[hh * 64:(hh + 1) * 64, :] = w_out[h * 64:(h + 1) * 64, :]
        in_maps.append({
            "x": np.ascontiguousarray(x[b]),
            "wqk": _to_fp32r(wqk_c),
            "wv": _to_fp32r(wv_c),
            "wout": _to_fp32r(wout_c),
            "cosT": cosT,
            "sinT2": sinT2,
            "ident": ident,
        })

    res = run_bass_kernel_spmd(nc, in_maps, core_ids=list(range(8))).results

    y = np.zeros((B, N, DIM), np.float32)
    v = np.zeros((B, HEADS, N, DHEAD), np.float32)
    for c in range(8):
        b, hg = c // 4, c % 4
        y[b] += res[c]["y"]
        vo = res[c]["vout"]  # [N, NH, DHEAD]
        for hh in range(NH):
            v[b, hg * NH + hh] = vo[:, hh, :]
    return y, v


# revision 8
# speedup vs baseline: 1.0707x; 1.0707x over previous
"""Block-causal attention kernel for Trainium2, 8 NeuronCores.

Sharding: core c in 0..7 handles batch b = c//4 and 4 heads starting at
(c%4)*4.  Each core computes RMSNorm, QKV projection (fp32r / TF32 matmuls),
interleaved RoPE, block-causal attention, and a partial output projection over
its 4 heads.  Host sums the 4 partial y per batch and stacks v.

Self-contained: hardcodes shapes from the problem spec.
"""
import sys

sys.path.insert(0, "/opt/trn_rl_repo")

import numpy as np

import concourse.bacc as bacc
import concourse.tile as tile
from concourse import mybir
from concourse.bass_utils import run_bass_kernel_spmd

B, N, DIM = 2, 2048, 1024
HEADS, DHEAD, BLOCK = 16, 64, 128
THETA = 10000.0
P = 128
ST = N // P          # 16 seq tiles
KT = DIM // P        # 8 contraction tiles
NH = HEADS // 8 * 2  # 4 heads per core (wait: 16 heads / 4 groups)
NH = 4
SCALE = DHEAD ** -0.5
EPS = float(np.finfo(np.float32).eps)

f32 = mybir.dt.float32
f32r = mybir.dt.float32r
AF = mybir.ActivationFunctionType

PAIR_SWAP = [m ^ 1 for m in range(32)]


def _to_fp32r(x: np.ndarray) -> np.ndarray:
    """Round fp32 to tf32 (11-bit mantissa) the way the PE consumes it."""
    u = np.ascontiguousarray(x, dtype=np.float32).view(np.uint32)
    return (((u.astype(np.uint64) + 0x800) & 0xFFFFF000).astype(np.uint32)).view(
        np.float32
    )


def _emit(nc, tc, ctx):
    from contextlib import ExitStack

    x_d = nc.declare_dram_parameter("x", [N, DIM], f32, isOutput=False)
    wqk_d = nc.declare_dram_parameter("wqk", [DIM, NH * 128], f32r, isOutput=False)
    wv_d = nc.declare_dram_parameter("wv", [DIM, NH * 64], f32r, isOutput=False)
    wout_d = nc.declare_dram_parameter("wout", [NH * 64, DIM], f32r, isOutput=False)
    cos_d = nc.declare_dram_parameter("cosT", [P, N], f32, isOutput=False)
    sin2_d = nc.declare_dram_parameter("sinT2", [P, N], f32, isOutput=False)
    id_d = nc.declare_dram_parameter("ident", [P, P], f32r, isOutput=False)
    y_d = nc.declare_dram_parameter("y", [N, DIM], f32, isOutput=True)
    v_d = nc.declare_dram_parameter("vout", [N, NH, DHEAD], f32, isOutput=True)

    persist = ctx.enter_context(tc.tile_pool(name="persist", bufs=1))
    v_sb = persist.tile([P, ST, NH, DHEAD + 1], f32r)
    qq = [persist.tile([P, N], f32r, tag=f"qq{p}", name=f"qq{p}") for p in range(2)]
    kk = [persist.tile([P, N], f32r, tag=f"kk{p}", name=f"kk{p}") for p in range(2)]
    eps_sb = persist.tile([P, 1], f32)
    nc.vector.memset(eps_sb[:], EPS)
    one_sb = persist.tile([P, 1], f32)
    nc.vector.memset(one_sb[:], 1.0)

    with ExitStack() as early:
        epool = early.enter_context(tc.tile_pool(name="epool", bufs=1))
        work = early.enter_context(tc.tile_pool(name="work", bufs=3))
        rope = early.enter_context(tc.tile_pool(name="rope", bufs=2))
        stat = early.enter_context(tc.tile_pool(name="stat", bufs=4))
        ps_tp = early.enter_context(tc.tile_pool(name="ps_tp", bufs=2, space="PSUM"))
        ps_qk = early.enter_context(tc.tile_pool(name="ps_qk", bufs=1, space="PSUM"))
        ps_v = early.enter_context(tc.tile_pool(name="ps_v", bufs=2, space="PSUM"))

        wqk_sb = epool.tile([P, KT, NH * 128], f32r)
        wv_sb = epool.tile([P, KT, NH * 64], f32r)
        cos_sb = epool.tile([P, N], f32)
        sin2_sb = epool.tile([P, N], f32)
        id_sb = epool.tile([P, P], f32r)
        xnT = epool.tile([P, KT, N], f32r)

        for kt in range(KT):
            nc.sync.dma_start(out=wqk_sb[:, kt, :], in_=wqk_d[kt * P:(kt + 1) * P, :])
            nc.sync.dma_start(out=wv_sb[:, kt, :], in_=wv_d[kt * P:(kt + 1) * P, :])
        nc.sync.dma_start(out=cos_sb[:], in_=cos_d[:])
        nc.sync.dma_start(out=sin2_sb[:], in_=sin2_d[:])
        nc.sync.dma_start(out=id_sb[:], in_=id_d[:])

        # ---- phase 1: rmsnorm + transpose -> xnT --------------------------
        for s in range(ST):
            x_t = work.tile([P, DIM], f32, tag="x")
            nc.sync.dma_start(out=x_t[:], in_=x_d[s * P:(s + 1) * P, :])
            sq = work.tile([P, DIM], f32, tag="sq", bufs=2)
            ss = stat.tile([P, 1], f32, tag="ss")
            nc.scalar.activation(out=sq[:], in_=x_t[:], func=AF.Square,
                                 accum_out=ss[:])
            rstd = stat.tile([P, 1], f32, tag="rstd")
            nc.scalar.activation(out=rstd[:], in_=ss[:], func=AF.Sqrt,
                                 bias=eps_sb[:], scale=1.0 / DIM)
            nc.vector.reciprocal(out=rstd[:], in_=rstd[:])
            xn = work.tile([P, DIM], f32r, tag="xn")
            nc.scalar.activation(out=xn[:], in_=x_t[:], func=AF.Copy,
                                 scale=rstd[:])
            for g in range(2):
                pst = ps_tp.tile([P, 4, P], f32r, tag="pst")
                for dd in range(4):
                    d = g * 4 + dd
                    nc.tensor.transpose(pst[:, dd, :], xn[:, d * P:(d + 1) * P],
                                        id_sb[:])
                nc.scalar.copy(
                    out=xnT[:, g * 4:(g + 1) * 4, s * P:(s + 1) * P], in_=pst[:])

        # ---- phase 2: v projection ---------------------------------------
        for s in range(ST):
            pv = ps_v.tile([P, NH * 64], f32, tag="pv")
            for kt in range(KT):
                nc.tensor.matmul(
                    pv[:], xnT[:, kt, s * P:(s + 1) * P], wv_sb[:, kt, :],
                    start=(kt == 0), stop=(kt == KT - 1))
            vo = work.tile([P, NH, DHEAD], f32, tag="vo", bufs=2)
            nc.scalar.copy(out=vo[:], in_=pv[:].rearrange("p (h d) -> p h d", h=NH))
            nc.sync.dma_start(out=v_d[s * P:(s + 1) * P, :, :], in_=vo[:])
            nc.vector.tensor_copy(
                out=v_sb[:, s, :, 0:DHEAD],
                in_=pv[:].rearrange("p (h d) -> p h d", h=NH))
            nc.vector.tensor_copy(out=v_sb[:, s, :, DHEAD],
                                  in_=one_sb[:].to_broadcast((P, NH)))

        # ---- phase 3: qk projection + rope -------------------------------
        for h in range(NH):
            ro = (h % 2) * 64
            pl = h // 2
            for sh in range(2):
                pqk = ps_qk.tile([P, N // 2], f32, tag="pqk")
                for kt in range(KT):
                    for nch in range(2):
                        c0 = sh * (N // 2) + nch * 512
                        nc.tensor.matmul(
                            pqk[:, nch * 512:(nch + 1) * 512],
                            wqk_sb[:, kt, h * 128:(h + 1) * 128],
                            xnT[:, kt, c0:c0 + 512],
                            start=(kt == 0), stop=(kt == KT - 1))
                for nch in range(2):
                    col = sh * (N // 2) + nch * 512
                    pq = pqk[:, nch * 512:(nch + 1) * 512]
                    a_t = rope.tile([P, 512], f32, tag="ropea")
                    nc.vector.tensor_mul(a_t[:], pq, sin2_sb[:, col:col + 512])
                    b_t = rope.tile([P, 512], f32, tag="ropeb")
                    nc.vector.stream_shuffle(b_t[:], a_t[:], PAIR_SWAP)
                    c_t = rope.tile([P, 512], f32, tag="ropec")
                    nc.vector.tensor_mul(c_t[:], pq, cos_sb[:, col:col + 512])
                    nc.vector.tensor_add(
                        qq[pl][ro:ro + 64, col:col + 512], c_t[0:64, :],
                        b_t[0:64, :])
                    nc.vector.tensor_add(
                        kk[pl][ro:ro + 64, col:col + 512], c_t[64:128, :],
                        b_t[64:128, :])

    # ---- phase 4: attention + phase 5: output projection ------------------
    with ExitStack() as late:
        lpool = late.enter_context(tc.tile_pool(name="lpool", bufs=1))
        expp = late.enter_context(tc.tile_pool(name="expp", bufs=4))
        lstat = late.enter_context(tc.tile_pool(name="lstat", bufs=2))
        yout = late.enter_context(tc.tile_pool(name="yout", bufs=2))
        ps_sim = late.enter_context(tc.tile_pool(name="ps_sim", bufs=3,
                                                 space="PSUM"))
        ps_o = late.enter_context(tc.tile_pool(name="ps_o", bufs=1, space="PSUM"))
        ps_y = late.enter_context(tc.tile_pool(name="ps_y", bufs=2, space="PSUM"))
        dscratch = late.enter_context(tc.tile_pool(name="dscratch", bufs=2,
                                                   space="DRAM"))

        aoT = lpool.tile([P, 2, N], f32r)
        wout_sb = lpool.tile([P, 2, DIM], f32r)
        for kp in range(2):
            nc.sync.dma_start(out=wout_sb[:, kp, :],
                              in_=wout_d[kp * P:(kp + 1) * P, :])

        for h in range(NH):
            ro = (h % 2) * 64
            pl = h // 2
            for half in range(2):
                po = ps_o.tile([DHEAD + 1, N // 2], f32, tag="po")
                for j in range(8 * half + 8):
                    cs = list(range(max(j // 4, 2 * half), 2 * half + 2))
                    psims, ets = {}, {}
                    for c in cs:
                        i0 = max(j * P, c * 512)
                        nw = (c + 1) * 512 - i0
                        psim = ps_sim.tile([P, 512], f32, tag="psim")
                        nc.tensor.matmul(
                            psim[:, 0:nw],
                            kk[pl][ro:ro + 64, j * P:(j + 1) * P],
                            qq[pl][ro:ro + 64, i0:i0 + nw],
                            start=True, stop=True)
                        psims[c] = psim
                    for c in cs:
                        i0 = max(j * P, c * 512)
                        nw = (c + 1) * 512 - i0
                        et = expp.tile([P, 512], f32r, tag="et")
                        nc.scalar.activation(out=et[:, 0:nw],
                                             in_=psims[c][:, 0:nw], func=AF.Exp)
                        ets[c] = et
                    for c in cs:
                        i0 = max(j * P, c * 512)
                        nw = (c + 1) * 512 - i0
                        nc.tensor.matmul(
                            po[:, i0 - half * (N // 2):i0 - half * (N // 2) + nw],
                            v_sb[:, j, h, :],
                            ets[c][:, 0:nw],
                            start=(j == 0), stop=(j == 4 * c + 3))
                s_sb = lstat.tile([1, N // 2], f32, tag="ssum")
                nc.scalar.copy(out=s_sb[:], in_=po[DHEAD:DHEAD + 1, :])
                r_sb = lstat.tile([1, N // 2], f32, tag="rsum")
                nc.vector.reciprocal_approx_fast(out=r_sb[:], in_=s_sb[:])
                r_dr = dscratch.tile([1, N // 2], f32, tag="rdr")
                nc.sync.dma_start(out=r_dr[:], in_=r_sb[:])
                bc = lstat.tile([64, N // 2], f32, tag="bcast")
                nc.sync.dma_start(out=bc[:],
                                  in_=r_dr[:].to_broadcast((64, N // 2)))
                nc.vector.tensor_mul(
                    aoT[ro:ro + 64, pl,
                        half * (N // 2):(half + 1) * (N // 2)],
                    po[0:DHEAD, :], bc[:])

        # ---- phase 5: output projection ----------------------------------
        for s in range(ST):
            ys = yout.tile([P, DIM], f32, tag="ys")
            for nh2 in range(2):
                py = ps_y.tile([P, 512], f32, tag="py")
                for kp in range(2):
                    nc.tensor.matmul(
                        py[:],
                        aoT[:, kp, s * P:(s + 1) * P],
                        wout_sb[:, kp, nh2 * 512:(nh2 + 1) * 512],
                        start=(kp == 0), stop=(kp == 1))
                nc.vector.tensor_copy(out=ys[:, nh2 * 512:(nh2 + 1) * 512],
                                      in_=py[:])
            nc.sync.dma_start(out=y_d[s * P:(s + 1) * P, :], in_=ys[:])


_NC = None


def _build():
    global _NC
    if _NC is None:
        from contextlib import ExitStack

        nc = bacc.Bacc("TRN2", target_bir_lowering=False, debug=False)
        with tile.TileContext(nc) as tc:
            with ExitStack() as ctx:
                _emit(nc, tc, ctx)
        nc.finalize()
        _NC = nc
    return _NC


def _host_tables():
    inv_freq = 1.0 / (THETA ** (np.arange(0, DHEAD, 2, dtype=np.float64) / DHEAD))
    pos = np.arange(N, dtype=np.float64)
    fr = pos[:, None] * inv_freq[None, :]          # [N, 32]
    fr = np.repeat(fr, 2, axis=-1)                 # [N, 64]
    cosd = np.cos(fr).T.astype(np.float32)         # [64, N]
    sind = np.sin(fr).T.astype(np.float32)
    sgn = np.where(np.arange(DHEAD) % 2 == 0, 1.0, -1.0).astype(np.float32)[:, None]
    cosT = np.concatenate([cosd * SCALE, cosd], axis=0)          # [128, N]
    sinT2 = np.concatenate([sind * sgn * SCALE, sind * sgn], axis=0)
    return cosT.astype(np.float32), sinT2.astype(np.float32)


def kernel(x, norm_w, w_qkv, w_out):
    x = np.asarray(x, dtype=np.float32)
    norm_w = np.asarray(norm_w, dtype=np.float32)
    w_qkv = np.asarray(w_qkv, dtype=np.float32)
    w_out = np.asarray(w_out, dtype=np.float32)

    nc = _build()
    cosT, sinT2 = _host_tables()
    ident = _to_fp32r(np.eye(P, dtype=np.float32))

    w_eff = w_qkv * norm_w[:, None]
    wq = w_eff[:, 0:HEADS * DHEAD]
    wk = w_eff[:, HEADS * DHEAD:2 * HEADS * DHEAD]
    wv = w_eff[:, 2 * HEADS * DHEAD:]

    in_maps = []
    for c in range(8):
        b, hg = c // 4, c % 4
        hs = [hg * NH + hh for hh in range(NH)]
        wqk_c = np.empty((DIM, NH * 128), np.float32)
        wv_c = np.empty((DIM, NH * 64), np.float32)
        wout_c = np.empty((NH * 64, DIM), np.float32)
        for hh, h in enumerate(hs):
            wqk_c[:, hh * 128:hh * 128 + 64] = wq[:, h * 64:(h + 1) * 64]
            wqk_c[:, hh * 128 + 64:hh * 128 + 128] = wk[:, h * 64:(h + 1) * 64]
            wv_c[:, hh * 64:(hh + 1) * 64] = wv[:, h * 64:(h + 1) * 64]
            wout_c[hh * 64:(hh + 1) * 64, :] = w_out[h * 64:(h + 1) * 64, :]
        in_maps.append({
            "x": np.ascontiguousarray(x[b]),
            "wqk": _to_fp32r(wqk_c),
            "wv": _to_fp32r(wv_c),
            "wout": _to_fp32r(wout_c),
            "cosT": cosT,
            "sinT2": sinT2,
            "ident": ident,
        })

    res = run_bass_kernel_spmd(nc, in_maps, core_ids=list(range(8))).results

    y = np.zeros((B, N, DIM), np.float32)
    v = np.zeros((B, HEADS, N, DHEAD), np.float32)
    for c in range(8):
        b, hg = c // 4, c % 4
        y[b] += res[c]["y"]
        vo = res[c]["vout"]  # [N, NH, DHEAD]
        for hh in range(NH):
            v[b, hg * NH + hh] = vo[:, hh, :]
    return y, v


# revision 9
# speedup vs baseline: 1.1946x; 1.1158x over previous
"""Block-causal attention kernel for Trainium2, 8 NeuronCores.

Sharding: core c in 0..7 handles batch b = c//4 and 4 heads starting at
(c%4)*4.  Each core computes RMSNorm, QKV projection (fp32r / TF32 matmuls),
interleaved RoPE, block-causal attention, and a partial output projection over
its 4 heads.  Host sums the 4 partial y per batch and stacks v.

Self-contained: hardcodes shapes from the problem spec.
"""
import sys

sys.path.insert(0, "/opt/trn_rl_repo")

import numpy as np

import concourse.bacc as bacc
import concourse.tile as tile
from concourse import mybir
from concourse.bass_utils import run_bass_kernel_spmd

B, N, DIM = 2, 2048, 1024
HEADS, DHEAD, BLOCK = 16, 64, 128
THETA = 10000.0
P = 128
ST = N // P          # 16 seq tiles
KT = DIM // P        # 8 contraction tiles
NH = HEADS // 8 * 2  # 4 heads per core (wait: 16 heads / 4 groups)
NH = 4
SCALE = DHEAD ** -0.5
EPS = float(np.finfo(np.float32).eps)

f32 = mybir.dt.float32
f32r = mybir.dt.float32r
AF = mybir.ActivationFunctionType

PAIR_SWAP = [m ^ 1 for m in range(32)]


def _to_fp32r(x: np.ndarray) -> np.ndarray:
    """Round fp32 to tf32 (11-bit mantissa) the way the PE consumes it."""
    u = np.ascontiguousarray(x, dtype=np.float32).view(np.uint32)
    return (((u.astype(np.uint64) + 0x800) & 0xFFFFF000).astype(np.uint32)).view(
        np.float32
    )


def _emit(nc, tc, ctx):
    from contextlib import ExitStack

    x_d = nc.declare_dram_parameter("x", [N, DIM], f32, isOutput=False)
    wqk_d = nc.declare_dram_parameter("wqk", [DIM, NH * 128], f32r, isOutput=False)
    wv_d = nc.declare_dram_parameter("wv", [DIM, NH * 64], f32r, isOutput=False)
    wout_d = nc.declare_dram_parameter("wout", [NH * 64, DIM], f32r, isOutput=False)
    cos_d = nc.declare_dram_parameter("cosT", [P, N], f32, isOutput=False)
    sin2_d = nc.declare_dram_parameter("sinT2", [P, N], f32, isOutput=False)
    id_d = nc.declare_dram_parameter("ident", [P, P], f32r, isOutput=False)
    y_d = nc.declare_dram_parameter("y", [N, DIM], f32, isOutput=True)
    v_d = nc.declare_dram_parameter("vout", [N, NH, DHEAD], f32, isOutput=True)

    persist = ctx.enter_context(tc.tile_pool(name="persist", bufs=1))
    v_sb = persist.tile([P, ST, NH, DHEAD + 1], f32r)
    qq = [persist.tile([P, N], f32r, tag=f"qq{p}", name=f"qq{p}") for p in range(2)]
    kk = [persist.tile([P, N], f32r, tag=f"kk{p}", name=f"kk{p}") for p in range(2)]
    eps_sb = persist.tile([P, 1], f32)
    nc.vector.memset(eps_sb[:], EPS)
    one_sb = persist.tile([P, 1], f32)
    nc.vector.memset(one_sb[:], 1.0)

    with ExitStack() as early:
        epool = early.enter_context(tc.tile_pool(name="epool", bufs=1))
        work = early.enter_context(tc.tile_pool(name="work", bufs=3))
        rope = early.enter_context(tc.tile_pool(name="rope", bufs=2))
        stat = early.enter_context(tc.tile_pool(name="stat", bufs=4))
        ps_tp = early.enter_context(tc.tile_pool(name="ps_tp", bufs=2, space="PSUM"))
        ps_qk = early.enter_context(tc.tile_pool(name="ps_qk", bufs=2, space="PSUM"))
        ps_v = early.enter_context(tc.tile_pool(name="ps_v", bufs=2, space="PSUM"))

        wqk_sb = epool.tile([P, KT, NH * 128], f32r)
        wv_sb = epool.tile([P, KT, NH * 64], f32r)
        cos_sb = epool.tile([P, N], f32)
        sin2_sb = epool.tile([P, N], f32)
        id_sb = epool.tile([P, P], f32r)
        xnT = epool.tile([P, KT, N], f32r)

        for kt in range(KT):
            nc.sync.dma_start(out=wqk_sb[:, kt, :], in_=wqk_d[kt * P:(kt + 1) * P, :])
            nc.sync.dma_start(out=wv_sb[:, kt, :], in_=wv_d[kt * P:(kt + 1) * P, :])
        nc.sync.dma_start(out=cos_sb[:], in_=cos_d[:])
        nc.sync.dma_start(out=sin2_sb[:], in_=sin2_d[:])
        nc.sync.dma_start(out=id_sb[:], in_=id_d[:])

        # ---- phase 1: rmsnorm + transpose -> xnT --------------------------
        for s in range(ST):
            x_t = work.tile([P, DIM], f32, tag="x")
            nc.sync.dma_start(out=x_t[:], in_=x_d[s * P:(s + 1) * P, :])
            sq = work.tile([P, DIM], f32, tag="sq", bufs=2)
            ss = stat.tile([P, 1], f32, tag="ss")
            nc.scalar.activation(out=sq[:], in_=x_t[:], func=AF.Square,
                                 accum_out=ss[:])
            rstd = stat.tile([P, 1], f32, tag="rstd")
            nc.scalar.activation(out=rstd[:], in_=ss[:], func=AF.Sqrt,
                                 bias=eps_sb[:], scale=1.0 / DIM)
            nc.vector.reciprocal(out=rstd[:], in_=rstd[:])
            xn = work.tile([P, DIM], f32r, tag="xn")
            nc.scalar.activation(out=xn[:], in_=x_t[:], func=AF.Copy,
                                 scale=rstd[:])
            for g in range(2):
                pst = ps_tp.tile([P, 4, P], f32r, tag="pst")
                for dd in range(4):
                    d = g * 4 + dd
                    nc.tensor.transpose(pst[:, dd, :], xn[:, d * P:(d + 1) * P],
                                        id_sb[:])
                nc.scalar.copy(
                    out=xnT[:, g * 4:(g + 1) * 4, s * P:(s + 1) * P], in_=pst[:])

        # ---- phase 2: v projection ---------------------------------------
        for s in range(ST):
            pv = ps_v.tile([P, NH * 64], f32, tag="pv")
            for kt in range(KT):
                nc.tensor.matmul(
                    pv[:], xnT[:, kt, s * P:(s + 1) * P], wv_sb[:, kt, :],
                    start=(kt == 0), stop=(kt == KT - 1))
            vo = work.tile([P, NH, DHEAD], f32, tag="vo", bufs=2)
            nc.scalar.copy(out=vo[:], in_=pv[:].rearrange("p (h d) -> p h d", h=NH))
            nc.sync.dma_start(out=v_d[s * P:(s + 1) * P, :, :], in_=vo[:])
            nc.vector.tensor_copy(
                out=v_sb[:, s, :, 0:DHEAD],
                in_=pv[:].rearrange("p (h d) -> p h d", h=NH))
            nc.vector.tensor_copy(out=v_sb[:, s, :, DHEAD],
                                  in_=one_sb[:].to_broadcast((P, NH)))

        # ---- phase 3: qk projection + rope -------------------------------
        for h in range(NH):
            ro = (h % 2) * 64
            pl = h // 2
            for sh in range(2):
                pqk = ps_qk.tile([P, N // 2], f32, tag="pqk")
                for kt in range(KT):
                    for nch in range(2):
                        c0 = sh * (N // 2) + nch * 512
                        nc.tensor.matmul(
                            pqk[:, nch * 512:(nch + 1) * 512],
                            wqk_sb[:, kt, h * 128:(h + 1) * 128],
                            xnT[:, kt, c0:c0 + 512],
                            start=(kt == 0), stop=(kt == KT - 1))
                for nch in range(2):
                    col = sh * (N // 2) + nch * 512
                    pq = pqk[:, nch * 512:(nch + 1) * 512]
                    a_t = rope.tile([P, 512], f32, tag="ropea")
                    nc.vector.tensor_mul(a_t[:], pq, sin2_sb[:, col:col + 512])
                    b_t = rope.tile([P, 512], f32, tag="ropeb")
                    nc.vector.stream_shuffle(b_t[:], a_t[:], PAIR_SWAP)
                    c_t = rope.tile([P, 512], f32, tag="ropec")
                    nc.vector.tensor_mul(c_t[:], pq, cos_sb[:, col:col + 512])
                    nc.vector.tensor_add(
                        qq[pl][ro:ro + 64, col:col + 512], c_t[0:64, :],
                        b_t[0:64, :])
                    nc.vector.tensor_add(
                        kk[pl][ro:ro + 64, col:col + 512], c_t[64:128, :],
                        b_t[64:128, :])

    # ---- phase 4: attention + phase 5: output projection ------------------
    with ExitStack() as late:
        lpool = late.enter_context(tc.tile_pool(name="lpool", bufs=1))
        expp = late.enter_context(tc.tile_pool(name="expp", bufs=4))
        lstat = late.enter_context(tc.tile_pool(name="lstat", bufs=2))
        yout = late.enter_context(tc.tile_pool(name="yout", bufs=2))
        ps_sim = late.enter_context(tc.tile_pool(name="ps_sim", bufs=2,
                                                 space="PSUM"))
        ps_o = late.enter_context(tc.tile_pool(name="ps_o", bufs=2, space="PSUM"))
        ps_y = late.enter_context(tc.tile_pool(name="ps_y", bufs=2, space="PSUM"))
        dscratch = late.enter_context(tc.tile_pool(name="dscratch", bufs=2,
                                                   space="DRAM"))

        aoT = lpool.tile([P, 2, N], f32r)
        wout_sb = lpool.tile([P, 2, DIM], f32r)
        for kp in range(2):
            nc.sync.dma_start(out=wout_sb[:, kp, :],
                              in_=wout_d[kp * P:(kp + 1) * P, :])

        for h in range(NH):
            ro = (h % 2) * 64
            pl = h // 2
            for half in range(2):
                po = ps_o.tile([DHEAD + 1, N // 2], f32, tag="po")
                for j in range(8 * half + 8):
                    cs = list(range(max(j // 4, 2 * half), 2 * half + 2))
                    psims, ets = {}, {}
                    for c in cs:
                        i0 = max(j * P, c * 512)
                        nw = (c + 1) * 512 - i0
                        psim = ps_sim.tile([P, 512], f32, tag="psim")
                        nc.tensor.matmul(
                            psim[:, 0:nw],
                            kk[pl][ro:ro + 64, j * P:(j + 1) * P],
                            qq[pl][ro:ro + 64, i0:i0 + nw],
                            start=True, stop=True)
                        psims[c] = psim
                    for c in cs:
                        i0 = max(j * P, c * 512)
                        nw = (c + 1) * 512 - i0
                        et = expp.tile([P, 512], f32r, tag="et")
                        nc.scalar.activation(out=et[:, 0:nw],
                                             in_=psims[c][:, 0:nw], func=AF.Exp)
                        ets[c] = et
                    for c in cs:
                        i0 = max(j * P, c * 512)
                        nw = (c + 1) * 512 - i0
                        nc.tensor.matmul(
                            po[:, i0 - half * (N // 2):i0 - half * (N // 2) + nw],
                            v_sb[:, j, h, :],
                            ets[c][:, 0:nw],
                            start=(j == 0), stop=(j == 4 * c + 3))
                s_sb = lstat.tile([1, N // 2], f32, tag="ssum")
                nc.scalar.copy(out=s_sb[:], in_=po[DHEAD:DHEAD + 1, :])
                r_sb = lstat.tile([1, N // 2], f32, tag="rsum")
                nc.vector.reciprocal_approx_fast(out=r_sb[:], in_=s_sb[:])
                r_dr = dscratch.tile([1, N // 2], f32, tag="rdr")
                nc.sync.dma_start(out=r_dr[:], in_=r_sb[:])
                bc = lstat.tile([64, N // 2], f32, tag="bcast")
                nc.sync.dma_start(out=bc[:],
                                  in_=r_dr[:].to_broadcast((64, N // 2)))
                nc.vector.tensor_mul(
                    aoT[ro:ro + 64, pl,
                        half * (N // 2):(half + 1) * (N // 2)],
                    po[0:DHEAD, :], bc[:])

        # ---- phase 5: output projection ----------------------------------
        for s in range(ST):
            ys = yout.tile([P, DIM], f32, tag="ys")
            for nh2 in range(2):
                py = ps_y.tile([P, 512], f32, tag="py")
                for kp in range(2):
                    nc.tensor.matmul(
                        py[:],
                        aoT[:, kp, s * P:(s + 1) * P],
                        wout_sb[:, kp, nh2 * 512:(nh2 + 1) * 512],
                        start=(kp == 0), stop=(kp == 1))
                nc.vector.tensor_copy(out=ys[:, nh2 * 512:(nh2 + 1) * 512],
                                      in_=py[:])
            nc.sync.dma_start(out=y_d[s * P:(s + 1) * P, :], in_=ys[:])


_NC = None


def _build():
    global _NC
    if _NC is None:
        from contextlib import ExitStack

        nc = bacc.Bacc("TRN2", target_bir_lowering=False, debug=False)
        with tile.TileContext(nc) as tc:
            with ExitStack() as ctx:
                _emit(nc, tc, ctx)
        nc.finalize()
        _NC = nc
    return _NC


def _host_tables():
    inv_freq = 1.0 / (THETA ** (np.arange(0, DHEAD, 2, dtype=np.float64) / DHEAD))
    pos = np.arange(N, dtype=np.float64)
    fr = pos[:, None] * inv_freq[None, :]          # [N, 32]
    fr = np.repeat(fr, 2, axis=-1)                 # [N, 64]
    cosd = np.cos(fr).T.astype(np.float32)         # [64, N]
    sind = np.sin(fr).T.astype(np.float32)
    sgn = np.where(np.arange(DHEAD) % 2 == 0, 1.0, -1.0).astype(np.float32)[:, None]
    cosT = np.concatenate([cosd * SCALE, cosd], axis=0)          # [128, N]
    sinT2 = np.concatenate([sind * sgn * SCALE, sind * sgn], axis=0)
    return cosT.astype(np.float32), sinT2.astype(np.float32)


def kernel(x, norm_w, w_qkv, w_out):
    x = np.asarray(x, dtype=np.float32)
    norm_w = np.asarray(norm_w, dtype=np.float32)
    w_qkv = np.asarray(w_qkv, dtype=np.float32)
    w_out = np.asarray(w_out, dtype=np.float32)

    nc = _build()
    cosT, sinT2 = _host_tables()
    ident = _to_fp32r(np.eye(P, dtype=np.float32))

    w_eff = w_qkv * norm_w[:, None]
    wq = w_eff[:, 0:HEADS * DHEAD]
    wk = w_eff[:, HEADS * DHEAD:2 * HEADS * DHEAD]
    wv = w_eff[:, 2 * HEADS * DHEAD:]

    in_maps = []
    for c in range(8):
        b, hg = c // 4, c % 4
        hs = [hg * NH + hh for hh in range(NH)]
        wqk_c = np.empty((DIM, NH * 128), np.float32)
        wv_c = np.empty((DIM, NH * 64), np.float32)
        wout_c = np.empty((NH * 64, DIM), np.float32)
        for hh, h in enumerate(hs):
            wqk_c[:, hh * 128:hh * 128 + 64] = wq[:, h * 64:(h + 1) * 64]
            wqk_c[:, hh * 128 + 64:hh * 128 + 128] = wk[:, h * 64:(h + 1) * 64]
            wv_c[:, hh * 64:(hh + 1) * 64] = wv[:, h * 64:(h + 1) * 64]
            wout_c[hh * 64:(hh + 1) * 64, :] = w_out[h * 64:(h + 1) * 64, :]
        in_maps.append({
            "x": np.ascontiguousarray(x[b]),
            "wqk": _to_fp32r(wqk_c),
            "wv": _to_fp32r(wv_c),
            "wout": _to_fp32r(wout_c),
            "cosT": cosT,
            "sinT2": sinT2,
            "ident": ident,
        })

    res = run_bass_kernel_spmd(nc, in_maps, core_ids=list(range(8))).results

    y = np.zeros((B, N, DIM), np.float32)
    v = np.zeros((B, HEADS, N, DHEAD), np.float32)
    for c in range(8):
        b, hg = c // 4, c % 4
        y[b] += res[c]["y"]
        vo = res[c]["vout"]  # [N, NH, DHEAD]
        for hh in range(NH):
            v[b, hg * NH + hh] = vo[:, hh, :]
    return y, v
